# revision 30
# baseline (speedup 1.0000x reference)
"""LlamaAttention (B=2, S=2048, H=4096, 32 q heads / 8 kv heads, RoPE, causal)
on 8 Trainium2 NeuronCores.

Sharding: data-parallel over batch (2) x tensor-parallel over heads (4).
Core c = b*4 + t handles batch b with q heads 8t..8t+7 and kv heads 2t..2t+1.
Each core computes a partial output y_c = attn_out_local @ wo_local^T
([S, H], fp32); the host sums the 4 TP partials per batch.

All matmuls bf16 inputs / fp32 PSUM accumulation. All weight/activation
DRAM tensors are pre-interleaved on the host so each DMA is a direct image
of its SBUF destination (>=8KB contiguous per partition -> near-peak DMA).

Per-core structure (v8):
  KV phase : startup DMAs split and ordered so the first matmul's inputs
             (wk/hs/wv halves) land in ~6us; cos/sin/wq/mask trail behind
             the compute. kT[d, s] per kv head (+ fused RoPE); v in
             natural [s, d] layout with a ones column (vAug) so the
             softmax denominator falls out of the AV matmul for free.
             Per tb: vA copies are emitted BEFORE the kT RoPE so the pv
             PSUM banks free early for the QA phase's Q-projection.
  QA phase : q-blocks processed DESCENDING (tb3 first) so (a) tb3's hs
             tiles are reused straight from the KV phase (no reload, no
             transition stall) and (b) the attention drained after the
             LAST Q-proj is the lightest block (tb0), minimizing the
             ACT-bound tail. Q-proj runs in 4 quarter-passes (2 heads,
             2 PSUM slots each); scores are computed into 1024-col
             two-bank PSUM kc-pair tiles (one exp + at most one mask op
             per pair — halves ACT instruction/semaphore traffic), with
             diagonal chunks trimmed to the causally-live q-columns.
             All attention work (scores/exp pairs, AV+normalize per
             head, transposes lagging one head, per-quarter RoPE) is a
             stream of fine stages drained evenly between Q-proj steps,
             so ScalarE exp work hides under TensorE matmuls, RoPE never
             bursts an engine queue, and PE transposes never wait on the
             DVE normalize.
  O phase  : wo halves reuse the wq half slots (each DMA starts as soon
             as that wq half is dead, overlapping the QA tail); oT
             chunks are re-gathered from the DRAM scratch into hs-pool
             slots (tb-descending, to match flush order); y accumulated
             per 128-token row block, t-loop descending so the first
             rows consumed are the first rows flushed.
"""
import sys

sys.path.insert(0, "/opt/trn_rl_repo")

import numpy as np
import ml_dtypes

BF16 = ml_dtypes.bfloat16

B, S, H = 2, 2048, 4096
NH, NKV, HD = 32, 8, 128
THETA = 10000.0
SCALE = 1.0 / float(np.sqrt(HD))

N_CORES = 8
TP = 4
NH_L = NH // TP        # 8 local q heads
NKV_L = NKV // TP      # 2 local kv heads
GRP_L = NH_L // NKV_L  # 4 q heads per local kv head
TOKB = 512
NKC = H // 128         # 32 contraction chunks
NTB = S // TOKB        # 4 token blocks
NQC = S // 128         # 16 token chunks
VSTRIDE = 132          # per-chunk stride in vAug (129 used, pad for alignment)
KHALF = NKC // 2

_NC_CACHE = {}


def _rope(nc, rp, psum, cos_sb, sinn_sb, tsl, outT, col0, f32):
    """RoPE on a [128(d), TOKB] fp32 PSUM block; writes bf16 to outT[:, col0:+TOKB].

    out[0:64]   = p[0:64]*cos - p[64:128]*sin
    out[64:128] = p[64:128]*cos + p[0:64]*sin
    (cos rows duplicated; sin table half-swapped on host: rows 0:64 = +sin,
    rows 64:128 = -sin; bf16 tables so every DVE op takes the 16-bit 2x
    fast path.)
    """
    bf16 = outT.dtype
    # ACT stages the projection PSUM to SBUF bf16 first: this frees the
    # projection PSUM bank after one fast ACT op, and makes every RoPE op
    # a pure-bf16 DVE op (2x fast path, ~0.4us) instead of a mixed
    # f32-PSUM/f16 op (~0.9-1.1us) that starved the attention chain.
    pb = rp.tile([128, TOKB], bf16, tag="pb")
    nc.scalar.copy(pb[:], psum[:])
    tcos = rp.tile([128, TOKB], bf16, tag="tcos")
    nc.vector.tensor_mul(tcos[:], pb[:], cos_sb[:, tsl])
    trs = rp.tile([128, TOKB], bf16, tag="trs")
    nc.vector.tensor_mul(trs[0:64, :], pb[64:128, :], sinn_sb[64:128, tsl])
    nc.vector.tensor_mul(trs[64:128, :], pb[0:64, :], sinn_sb[0:64, tsl])
    nc.vector.tensor_add(outT[:, col0: col0 + TOKB], tcos[:], trs[:])


def _build(reps=1):
    import concourse.mybir as mybir
    import concourse.tile as tile
    from concourse import bacc
    from contextlib import ExitStack

    dt = mybir.dt
    f32, bf16, f16 = dt.float32, dt.bfloat16, dt.float16
    af = mybir.ActivationFunctionType

    nc = bacc.Bacc("TRN2", target_bir_lowering=False, debug=False,
                   enable_asserts=True, num_devices=N_CORES)
    # all pre-interleaved on host: DMA source rows == SBUF partition images
    hs_d = nc.dram_tensor("hs", [NTB * 128, NKC * TOKB], bf16, kind="ExternalInput").ap()
    wq_d = nc.dram_tensor("wq", [128, NKC * NH_L * 128], bf16, kind="ExternalInput").ap()
    wk_d = nc.dram_tensor("wk", [128, NKC * NKV_L * 128], bf16, kind="ExternalInput").ap()
    wv_d = nc.dram_tensor("wv", [128, NKC * NKV_L * 128], bf16, kind="ExternalInput").ap()
    wo_d = nc.dram_tensor("wo", [128, NH_L * H], bf16, kind="ExternalInput").ap()
    cos_d = nc.dram_tensor("cosT", [128, S], bf16, kind="ExternalInput").ap()
    sin_d = nc.dram_tensor("sinN", [128, S], bf16, kind="ExternalInput").ap()
    msk_d = nc.dram_tensor("maskB", [128, 4 * TOKB], bf16, kind="ExternalInput").ap()
    id_d = nc.dram_tensor("ident", [128, 128], bf16, kind="ExternalInput").ap()
    y_d = nc.dram_tensor("y", [S, H], f32, kind="ExternalOutput").ap()

    def emit(ctx, tc):
        # PSUM: ps = 4 x 2KB slots (Q-proj quarters, AV/transpose/KV-V/O),
        # psc = 2 x 4KB two-bank slots (1024-col score pairs, KV-K).
        ps = ctx.enter_context(tc.tile_pool(name="ps", bufs=4, space="PSUM"))
        psc = ctx.enter_context(tc.tile_pool(name="psc", bufs=2, space="PSUM"))
        persist = ctx.enter_context(tc.tile_pool(name="persist", bufs=1))
        dram = ctx.enter_context(tc.tile_pool(name="dram", bufs=1, space="DRAM"))

        mask_sb = persist.tile([128, 4 * TOKB], bf16, tag="mask")
        id_sb = persist.tile([128, 128], bf16, tag="ident")
        kT = persist.tile([128, NKV_L * S], bf16, tag="kT")
        vA = persist.tile([128, NKV_L * NQC * VSTRIDE], bf16, tag="vA")
        nc.gpsimd.memset(vA[:], 1.0)
        # oT scratch in HBM: col (tb*4096 + h*512 + s2*128)
        oT_dram = dram.tile([128, NH_L * S], bf16, tag="oTd")

        with tc.tile_pool(name="cs", bufs=1) as csp, \
             tc.tile_pool(name="wq", bufs=1) as wqp, \
             tc.tile_pool(name="hs", bufs=3) as hsp:
            cos_sb = csp.tile([128, S], bf16, tag="cos")
            sinn_sb = csp.tile([128, S], bf16, tag="sinn")
            # wq in two kc-halves so the O phase's wo can reuse each slot
            # as soon as that half's last Q-proj read retires.
            WQH = KHALF * NH_L * 128
            wqA = wqp.tile([128, WQH], bf16, tag="wqA")
            wqB = wqp.tile([128, WQH], bf16, tag="wqB")

            # DMAs trailing the KV phase's critical first loads; issued
            # between the hs half-tile DMAs inside emit_kv.
            late = {
                (0, 1): [lambda: nc.sync.dma_start(cos_sb[:], cos_d[:]),
                         lambda: nc.sync.dma_start(sinn_sb[:], sin_d[:])],
                (1, 1): [lambda: nc.sync.dma_start(wqA[:], wq_d[:, 0:WQH]),
                         lambda: nc.sync.dma_start(wqB[:], wq_d[:, WQH:]),
                         lambda: nc.sync.dma_start(mask_sb[:], msk_d[:]),
                         lambda: nc.sync.dma_start(id_sb[:], id_d[:])],
            }
            with tc.tile_pool(name="rope", bufs=2) as rp:
                hts3 = emit_kv(tc, ps, psc, rp, cos_sb, sinn_sb, kT, vA, hsp, late)
                emit_qa(tc, ps, psc, rp, cos_sb, sinn_sb, kT, vA,
                        mask_sb, id_sb, oT_dram, (wqA, wqB), wqp, hsp, hts3)
            emit_o_compute(tc, ps)

    def emit_kv(tc, ps, psc, rp, cos_sb, sinn_sb, kT, vA, hsp, late):
        """K/V projection + RoPE(K). Returns tb3's hs tiles for QA reuse."""
        with tc.tile_pool(name="wkv", bufs=1) as wkvp:
            # wk / first-hs / wv split into halves and interleaved so the
            # first K-proj matmul's inputs land in ~6us instead of ~20us.
            HW2 = KHALF * NKV_L * 128
            wk_sb = wkvp.tile([128, NKC * NKV_L * 128], bf16, tag="wk")
            nc.sync.dma_start(wk_sb[:, 0:HW2], wk_d[:, 0:HW2])
            wv_sb = wkvp.tile([128, NKC * NKV_L * 128], bf16, tag="wv")
            hts3 = None
            for tb in range(NTB):
                tsl = slice(tb * TOKB, (tb + 1) * TOKB)
                pks = [psc.tile([128, TOKB], f32, tag="sc", name=f"pk{tb}_{i}")
                       for i in range(NKV_L)]
                pvs = [ps.tile([128, 256], f32, tag="ps", name=f"pv{tb}_{i}")
                       for i in range(4)]
                hts = []
                for kh in range(2):
                    ht = hsp.tile([128, KHALF * TOKB], bf16, tag="hs",
                                  name=f"hskv{tb}_{kh}")
                    rows = hs_d[tb * 128:(tb + 1) * 128,
                                kh * KHALF * TOKB:(kh + 1) * KHALF * TOKB]
                    if tb == 0 and kh == 0:
                        # interleave: [hs.a, wv.a, wk.b, hs.b, wv.b] behind
                        # the already-issued wk.a.
                        HH = KHALF * TOKB // 2
                        nc.sync.dma_start(ht[:, 0:HH], rows[:, 0:HH])
                        nc.sync.dma_start(wv_sb[:, 0:HW2], wv_d[:, 0:HW2])
                        nc.sync.dma_start(wk_sb[:, HW2:], wk_d[:, HW2:])
                        nc.sync.dma_start(ht[:, HH:], rows[:, HH:])
                        nc.sync.dma_start(wv_sb[:, HW2:], wv_d[:, HW2:])
                    else:
                        nc.sync.dma_start(ht[:], rows)
                    hts.append(ht)
                    for cb in late.pop((tb, kh), []):
                        cb()
                for kh in range(2):
                    ht = hts[kh]
                    for k2 in range(KHALF):
                        kc = kh * KHALF + k2
                        hsl = slice(k2 * TOKB, (k2 + 1) * TOKB)
                        for g in range(NKV_L):
                            c0 = kc * 256 + g * 128
                            nc.tensor.matmul(
                                pks[g][:], wk_sb[:, c0:c0 + 128], ht[:, hsl],
                                start=(kc == 0), stop=(kc == NKC - 1))
                        for s in range(4):
                            nc.tensor.matmul(
                                pvs[s][:],
                                ht[:, k2 * TOKB + s * 128: k2 * TOKB + (s + 1) * 128],
                                wv_sb[:, kc * 256:(kc + 1) * 256],
                                start=(kc == 0), stop=(kc == NKC - 1))
                # vA copies first: frees the 4 pv PSUM banks before the
                # RoPE drains the pk banks (QA's Q-proj needs 4 banks).
                for s in range(4):
                    qc = tb * 4 + s
                    for g in range(NKV_L):
                        c0 = (g * NQC + qc) * VSTRIDE
                        nc.vector.tensor_copy(
                            vA[:, c0:c0 + 128],
                            pvs[s][:, g * 128:(g + 1) * 128])
                for g in range(NKV_L):
                    _rope(nc, rp, pks[g], cos_sb, sinn_sb, tsl, kT, g * S + tb * TOKB, f32)
                if tb == NTB - 1:
                    hts3 = hts
            return hts3

    def emit_qa(tc, ps, psc, rp, cos_sb, sinn_sb, kT, vA, mask_sb, id_sb,
                oT_dram, wq_halves, wqp, hsp, hts3):
        wqA, wqB = wq_halves
        with tc.tile_pool(name="qtb", bufs=2) as qp, \
             tc.tile_pool(name="otb", bufs=2) as op, \
             tc.tile_pool(name="exp", bufs=13) as ep, \
             tc.tile_pool(name="on", bufs=11) as onp:

            def attn_stages(tb):
                """Attention for q-block tb as a list of emission closures.

                Fine-grained: scores+exp in 1024-col kc-PAIR tiles (one ACT
                exp + at most one mask mul per pair — halves the ACT
                instruction/semaphore traffic), AV+normalize per head, PE
                transposes lagging one head so they never wait on the DVE
                normalize chain.
                """
                qTb = attn_stages.qtb[tb]
                oTb = op.tile([128, NH_L * TOKB], bf16, tag="oTb",
                              name=f"oTb{tb}")
                nkc = 4 * tb + 4
                npair = nkc // 2
                state = {}
                ons = {}

                def s1(h, pi0):
                    g = h // GRP_L
                    exps = state.setdefault(h, [])
                    for pi in range(pi0, min(pi0 + 2, npair)):
                        sc = psc.tile([128, 2 * TOKB], f32, tag="sc")
                        for sub in range(2):
                            kc = 2 * pi + sub
                            # diagonal chunk kc=4tb+r: q-cols < r*128 are
                            # never read by AV (strictly-upper chunks are
                            # skipped; the within-chunk triangle is handled
                            # by the 0/1 mask) — start the matmul at r*128.
                            r = kc - 4 * tb
                            q0 = r * 128 if r > 0 else 0
                            nc.tensor.matmul(
                                sc[:, sub * TOKB + q0:(sub + 1) * TOKB],
                                kT[:, g * S + kc * 128: g * S + (kc + 1) * 128],
                                qTb[:, h * TOKB + q0:(h + 1) * TOKB],
                                start=True, stop=True)
                        e = ep.tile([128, 2 * TOKB], bf16)
                        nc.scalar.activation(e[:], sc[:], af.Exp, scale=SCALE)
                        rel = pi - 2 * tb
                        if rel >= 0:  # diagonal band: multiplicative 0/1 mask
                            nc.vector.tensor_mul(
                                e[:], e[:],
                                mask_sb[:, rel * 2 * TOKB:(rel + 1) * 2 * TOKB])
                        exps.append(e)

                def s2(h):
                    g = h // GRP_L
                    exps = state.pop(h)
                    for s2i in range(4):
                        qc = 4 * tb + s2i
                        po = ps.tile([128, VSTRIDE], f32, tag="ps")
                        for kc in range(qc + 1):
                            c0 = (g * NQC + kc) * VSTRIDE
                            pi, sub = divmod(kc, 2)
                            nc.tensor.matmul(
                                po[:, 0:129],
                                exps[pi][:, sub * TOKB + s2i * 128:
                                        sub * TOKB + (s2i + 1) * 128],
                                vA[:, c0:c0 + 129],
                                start=(kc == 0), stop=(kc == qc))
                        rcp = onp.tile([128, 1], f32, tag="rcp")
                        nc.vector.reciprocal(rcp[:], po[:, 128:129])
                        on = onp.tile([128, 128], bf16, tag="on",
                                      name=f"on{tb}_{h}_{s2i}")
                        nc.vector.tensor_scalar_mul(on[:], po[:, 0:128], rcp[:])
                        ons[(h, s2i)] = on

                def trans(h):
                    for s2i in range(4):
                        on = ons.pop((h, s2i))
                        pt = ps.tile([128, 128], bf16, tag="ps")
                        nc.tensor.transpose(pt[:], on[:], id_sb[:])
                        nc.vector.tensor_copy(
                            oTb[:, h * TOKB + s2i * 128: h * TOKB + (s2i + 1) * 128],
                            pt[:])

                def flush():
                    nc.sync.dma_start(
                        oT_dram[:, tb * NH_L * TOKB:(tb + 1) * NH_L * TOKB],
                        oTb[:])

                stages = []
                for h in range(NH_L):
                    for pi0 in range(0, npair, 2):
                        stages.append(lambda h=h, pi0=pi0: s1(h, pi0))
                    if h >= 1:
                        stages.append(lambda h=h - 1: s2(h))
                    if h >= 2:
                        stages.append(lambda h=h - 2: trans(h))
                stages.append(lambda: s2(NH_L - 1))
                stages.append(lambda: trans(NH_L - 2))
                stages.append(lambda: trans(NH_L - 1))
                stages.append(flush)
                return stages

            attn_stages.qtb = {}

            def emit_q(tb, pending, reuse_hts=None):
                """Q projection for tb; `pending` = attention stages for
                tb+1 (descending order), drained adaptively between PSUM
                kc-chunks."""
                tsl = slice(tb * TOKB, (tb + 1) * TOKB)
                if reuse_hts is not None:
                    hts = reuse_hts
                else:
                    hts = []
                    for kh in range(2):
                        ht = hsp.tile([128, KHALF * TOKB], bf16, tag="hs",
                                      name=f"hsq{tb}_{kh}")
                        nc.sync.dma_start(
                            ht[:], hs_d[tb * 128:(tb + 1) * 128,
                                        kh * KHALF * TOKB:(kh + 1) * KHALF * TOKB])
                        hts.append(ht)
                qTb = qp.tile([128, NH_L * TOKB], bf16, tag="qTb",
                              name=f"qTb{tb}")
                attn_stages.qtb[tb] = qTb
                # 4 quarter-passes of 2 heads (2 PSUM slots each), 128
                # steps total; pending stages drained evenly between steps.
                # Each quarter's RoPE is pushed to the FRONT of `pending`
                # so it drains interleaved like any other stage instead of
                # forming an engine burst at the pass boundary.
                step = 0
                for qtr in range(4):
                    iv = max(1, (128 - step) // (len(pending) + 1)) \
                        if pending else 128
                    pqs = [ps.tile([128, TOKB], f32, tag="ps",
                                   name=f"pq{tb}_{qtr}_{i}") for i in range(2)]
                    for kc in range(NKC):
                        ht = hts[kc // KHALF]
                        hsl = slice((kc % KHALF) * TOKB, (kc % KHALF + 1) * TOKB)
                        wq_sb = wqA if kc < KHALF else wqB
                        for i in range(2):
                            h = qtr * 2 + i
                            c0 = (kc % KHALF) * 1024 + h * 128
                            nc.tensor.matmul(
                                pqs[i][:], wq_sb[:, c0:c0 + 128], ht[:, hsl],
                                start=(kc == 0), stop=(kc == NKC - 1))
                        step += 1
                        if step % iv == 0 and pending:
                            pending.pop(0)()

                    def rope_stage(pqs=pqs, qtr=qtr):
                        for i in range(2):
                            h = qtr * 2 + i
                            _rope(nc, rp, pqs[i], cos_sb, sinn_sb, tsl, qTb,
                                  h * TOKB, f32)
                    pending.insert(0, rope_stage)

            pending = []
            for tb in range(NTB - 1, -1, -1):
                emit_q(tb, pending, reuse_hts=hts3 if tb == NTB - 1 else None)
                pending = pending + attn_stages(tb)

            # O-phase DMAs issued before the final drain: wo halves reuse
            # the wq half slots (WAR on each half's last Q-proj read);
            # oT re-gather into hs-pool slots, tb-descending to match the
            # flush order (tb0's chunks last — its flush is in `pending`).
            WOH = (NH_L // 2) * H
            woA = wqp.tile([128, WOH], bf16, tag="wqA", name="woA")
            nc.sync.dma_start(woA[:], wo_d[:, 0:WOH])
            woB = wqp.tile([128, WOH], bf16, tag="wqB", name="woB")
            nc.sync.dma_start(woB[:], wo_d[:, WOH:])
            oTdcA = hsp.tile([128, KHALF * TOKB], bf16, tag="hs", name="oTdcA")
            oTdcB = hsp.tile([128, KHALF * TOKB], bf16, tag="hs", name="oTdcB")
            emit_o_compute.oTdc = (oTdcA, oTdcB)
            emit_o_compute.wo_sb = (woA, woB)

            def gather(tb):
                # oTdc{A,B} col layout: (dc%4)*S + tb*TOKB + s2*128
                for dc in range(NH_L):
                    dst = emit_o_compute.oTdc[dc // 4]
                    d0 = (dc % 4) * S + tb * TOKB
                    nc.sync.dma_start(
                        dst[:, d0: d0 + TOKB],
                        oT_dram[:, tb * NH_L * TOKB + dc * TOKB:
                                tb * NH_L * TOKB + (dc + 1) * TOKB])

            for tb in range(NTB - 1, 0, -1):
                gather(tb)
            for st in pending:
                st()
            gather(0)

    def emit_o_compute(tc, ps):
        oTdcA, oTdcB = emit_o_compute.oTdc
        woA, woB = emit_o_compute.wo_sb
        with tc.tile_pool(name="yrow", bufs=2) as yp:
            # descending t: tb3's oT chunks were flushed+gathered first
            for t in range(NQC - 1, -1, -1):
                yr = yp.tile([128, H], f32, tag="yr", name=f"yr{t}")
                for hb in range(H // 512):
                    py = ps.tile([128, 512], f32, tag="ps")
                    for dc in range(NH_L):
                        src = oTdcA if dc < 4 else oTdcB
                        d0 = (dc % 4) * S + t * 128
                        wo_sb = woA if dc < 4 else woB
                        c0 = (dc % 4) * H + hb * 512
                        nc.tensor.matmul(
                            py[:],
                            src[:, d0: d0 + 128],
                            wo_sb[:, c0: c0 + 512],
                            start=(dc == 0), stop=(dc == NH_L - 1))
                    nc.scalar.copy(yr[:, hb * 512:(hb + 1) * 512], py[:])
                nc.sync.dma_start(y_d[t * 128:(t + 1) * 128, :], yr[:])

    with tile.TileContext(nc) as tc:
        if reps == 1:
            with ExitStack() as ctx:
                emit(ctx, tc)
        else:
            with tc.For_i(0, reps, 1):
                with ExitStack() as ctx:
                    emit(ctx, tc)
    nc.compile()
    return nc


def get_nc(reps=1):
    if reps not in _NC_CACHE:
        _NC_CACHE[reps] = _build(reps)
    return _NC_CACHE[reps]


def make_in_maps(hidden_states, position_ids, wq, wk, wv, wo):
    hidden_states = np.asarray(hidden_states, dtype=np.float32)
    position_ids = np.asarray(position_ids)
    wq = np.asarray(wq, dtype=np.float32)
    wk = np.asarray(wk, dtype=np.float32)
    wv = np.asarray(wv, dtype=np.float32)
    wo = np.asarray(wo, dtype=np.float32)

    j = np.arange(64, dtype=np.float64)
    invf = 1.0 / (THETA ** (2.0 * j / HD))       # [64]
    kp = np.arange(128)[:, None]
    qf = np.arange(TOKB)[None, :]
    maskB = np.empty((128, 4 * TOKB), dtype=BF16)
    for r in range(4):
        maskB[:, r * TOKB:(r + 1) * TOKB] = (qf >= kp + 128 * r).astype(BF16)
    ident = np.eye(128, dtype=BF16)

    def interleave(wT, n):
        # [H_in, n] fp32 -> [128, (H_in/128)*n] bf16; row p holds all
        # contraction chunks for partition p (direct SBUF image)
        hin = wT.shape[0]
        return np.ascontiguousarray(
            wT.reshape(hin // 128, 128, n).transpose(1, 0, 2).reshape(128, -1)
        ).astype(BF16)

    in_maps = []
    for c in range(N_CORES):
        b, t = divmod(c, TP)
        pos = position_ids[b].astype(np.float64)     # [S]
        freqs = pos[:, None] * invf[None, :]         # [S, 64]
        cos64 = np.cos(freqs).astype(BF16).T         # [64, S]
        sin64 = np.sin(freqs).astype(BF16).T
        cosT = np.ascontiguousarray(np.concatenate([cos64, cos64], axis=0))
        # half-swapped for GpSimd RoPE: rows 0:64 = +sin, rows 64:128 = -sin
        sinN = np.ascontiguousarray(np.concatenate([sin64, -sin64], axis=0))

        hsT = hidden_states[b].T                     # [H, S] fp32
        # rows (tb*128 + p), cols (kc*512 + c)
        hs_i = np.ascontiguousarray(
            hsT.reshape(NKC, 128, NTB, TOKB).transpose(2, 1, 0, 3)
            .reshape(NTB * 128, NKC * TOKB)).astype(BF16)

        in_maps.append({
            "hs": hs_i,
            "wq": interleave(
                np.ascontiguousarray(wq[t * NH_L * HD:(t + 1) * NH_L * HD, :].T),
                NH_L * HD),
            "wk": interleave(
                np.ascontiguousarray(wk[t * NKV_L * HD:(t + 1) * NKV_L * HD, :].T),
                NKV_L * HD),
            "wv": interleave(
                np.ascontiguousarray(wv[t * NKV_L * HD:(t + 1) * NKV_L * HD, :].T),
                NKV_L * HD),
            "wo": interleave(
                np.ascontiguousarray(wo[:, t * NH_L * HD:(t + 1) * NH_L * HD].T), H),
            "cosT": cosT,
            "sinN": sinN,
            "maskB": maskB,
            "ident": ident,
        })
    return in_maps


def gather_out(results):
    """results: list of 8 dicts with 'y' [S, H] fp32 -> [B, S, H] fp32."""
    out = np.zeros((B, S, H), dtype=np.float32)
    for c in range(N_CORES):
        b = c // TP
        out[b] += results[c]["y"]
    return out


def kernel(**inputs):
    from concourse.bass_utils import run_bass_kernel_spmd

    nc = get_nc(reps=1)
    in_maps = make_in_maps(**inputs)
    res = run_bass_kernel_spmd(nc, in_maps, core_ids=list(range(N_CORES)))
    return gather_out(res.results)
